# revision 21
# baseline (speedup 1.0000x reference)
"""Differentiable Bezier path renderer on 8 Trainium2 NeuronCores.

Strategy (v4)
-------------
The reference rasterizes M=2048 path edges into a 512x512 soft
winding-number image:

    wind[h, w] = sum_e coeff(e, h) * sigmoid(x_cross(e, h) - w)
    alpha      = sigmoid(4 * wind),  rgb = broadcast(color)

Only (edge, row) pairs with t in [-TB, 1+TB] matter (~35k of 1M), and
per pair only a ~16px sigmoid transition window needs evaluation; left
of the window the pair contributes exactly coeff, right of it zero.

The host enumerates active pairs, computes their two scalars (coeff,
window-relative x_cross), assigns rows to cores minimizing the total
128-slot block count (64 rows/core, no collectives), buckets pairs
into 32px-aligned streams s (transition inside cols [32s, 32s+48)),
and packs everything into TWO fp16 blobs per core (DMA issue costs
~0.6us sequencer time each, so few big DMAs beat many small ones):
  blobA = xcf (fp32 bitcast) | -k iota | ls (fp32 bitcast)
          | w2 for the first blocks;   blobB = w2 for the rest
  w2[p, j*64 + r] = coeff_p * [row_p == r]  (one-hot scatter, fp16)
  xcf[p, j]       = x_cross_p - 32*s_p
  ls[r, b]        = sum of coeff over pairs with row r, s > b

Device per block j (slots on partitions), engines pipelined:
  * DVE    : ARG[p, jk] = xcf[p,j] - k        (one batched op/group)
  * ScalarE: SIG = sigmoid(ARG)               (one batched op/group)
  * TensorE: wind[r, 32s+k] += w2_j.T @ SIG_j  (fp16, 1 cyc/col, psum
             accumulation at absolute columns; 4 quarter banks; one
             start=True per bank clears its has_written bits, later
             matmuls overwrite-fresh/accumulate-written per cell)
Streams run right-to-left so each 128-col quarter finalizes early
(VectorE adds the broadcast 32px-block left sums in psum, ScalarE
writes alpha = sigmoid(4 wind) into a per-half staging tile); each
256-col half DMAs out as soon as its two quarters are done, while
matmuls for the left half continue.  The host assembles rgb = color
and re-orders the 8 per-core row sets.
"""

import numpy as np

import concourse.bacc as bacc
import concourse.mybir as mybir
import concourse.tile as tile
from concourse.bass_utils import run_bass_kernel_spmd

H = 512
W = 512
S = 64          # cubic bezier segments
TSAMP = 32      # samples per segment
M = S * TSAMP   # path points == edges
NCORES = 8
RPC = H // NCORES  # rows per core
NSTREAM = 16       # 32px-aligned window streams
A = 32             # stream alignment
SW = 48            # sigmoid window columns per pair
C = 8.0            # sigmoid cutoff (err ~ 0.24*exp(-C) per pixel)
TB = np.float32(0.45)  # t-window bound
NBA = 12           # blocks shipped in blobA (early pipeline start)
DT = mybir.dt.float32
F16 = mybir.dt.float16
AF = mybir.ActivationFunctionType

_prog_cache = {}


def _sigmoid64(z):
    with np.errstate(over="ignore", under="ignore"):
        return 1.0 / (1.0 + np.exp(-z.astype(np.float64)))


def _host_prep(control_points):
    """Sample the path, enumerate active (edge, row) pairs, assign rows to
    cores, bucket pairs into streams, pack 128-slot blocks into blobs.

    Returns (per_core_inputs, core_rows, nbs)."""
    cp = np.asarray(control_points, dtype=np.float32)
    p0 = cp[0:3 * S:3][:, None, :]
    p1 = cp[1:3 * S:3][:, None, :]
    p2 = cp[2:3 * S:3][:, None, :]
    p3 = cp[3:3 * S + 1:3][:, None, :]
    t = np.linspace(0.0, 1.0, TSAMP, dtype=np.float32)[None, :, None]
    mt = np.float32(1.0) - t
    pts = (mt ** 3) * p0 + 3.0 * (mt ** 2) * t * p1 \
        + 3.0 * mt * (t ** 2) * p2 + (t ** 3) * p3
    path = pts.reshape(-1, 2).astype(np.float32)

    nxt = np.roll(path, -1, axis=0)
    x0 = path[:, 0]
    y0 = path[:, 1]
    dy = nxt[:, 1] - y0
    dxe = nxt[:, 0] - x0
    dys = (dy + np.float32(1e-8)).astype(np.float32)
    recip = (np.float32(1.0) / dys).astype(np.float32)
    sm = (np.sign(dy) * (np.abs(dy) >= np.float32(1e-6))).astype(np.float32)

    g1 = y0 + (-TB) * dys
    g2 = y0 + (np.float32(1.0) + TB) * dys
    rlo = np.maximum(np.ceil(np.minimum(g1, g2)), 0.0).astype(np.int64)
    rhi = np.minimum(np.floor(np.maximum(g1, g2)), H - 1).astype(np.int64)
    act = (sm != 0) & (rhi >= rlo)
    eact = np.nonzero(act)[0]
    counts = (rhi[eact] - rlo[eact] + 1).astype(np.int64)
    pair_edge = np.repeat(eact, counts)
    pair_row = np.concatenate(
        [np.arange(rlo[e], rhi[e] + 1, dtype=np.int64) for e in eact]
    ) if len(eact) else np.zeros(0, np.int64)

    tval = ((pair_row.astype(np.float32) - y0[pair_edge]) * recip[pair_edge])
    cf = (_sigmoid64(20.0 * tval) * _sigmoid64(20.0 * (1.0 - tval))
          * sm[pair_edge]).astype(np.float32)
    xcv = (x0[pair_edge] + tval * dxe[pair_edge]).astype(np.float32)

    keep = xcv >= -C   # pairs entirely left of the image contribute ~0
    pair_row = pair_row[keep]
    cf = cf[keep]
    xcv = xcv[keep]

    seg = np.clip(np.floor((xcv - C) / A), 0, NSTREAM - 1).astype(np.int64)
    xcf = np.clip(xcv - A * seg.astype(np.float32), -60.0, 60.0)

    # Row -> core assignment minimizing the padded block count.
    rowcnt = np.bincount(pair_row, minlength=H)
    row_seg_cnt = np.zeros((H, NSTREAM), np.int64)
    np.add.at(row_seg_cnt, (pair_row, seg), 1)
    order = np.argsort(-rowcnt, kind="stable")
    core_rows = [[] for _ in range(NCORES)]
    loads = np.zeros(NCORES, np.int64)
    core_seg = np.zeros((NCORES, NSTREAM), np.int64)
    seg_max = np.zeros(NSTREAM, np.int64)
    for r in order:
        avail = [c for c in range(NCORES) if len(core_rows[c]) < RPC]
        best, bkey = None, None
        for c in avail:
            newmax = np.maximum(seg_max, core_seg[c] + row_seg_cnt[r])
            key = (int(newmax.sum()), int(loads[c]))
            if bkey is None or key < bkey:
                bkey, best = key, c
        c = best
        core_rows[c].append(int(r))
        loads[c] += rowcnt[r]
        core_seg[c] += row_seg_cnt[r]
        seg_max = np.maximum(seg_max, core_seg[c])
    row_core = np.empty(H, np.int64)
    row_loc = np.empty(H, np.int64)
    for c in range(NCORES):
        for i, r in enumerate(core_rows[c]):
            row_core[r] = c
            row_loc[r] = i

    pair_core = row_core[pair_row]
    nbs = [max(1, int(np.ceil(seg_max[s] / 128.0))) for s in range(NSTREAM)]
    total = sum(nbs)
    pad = (-total) % 2
    nbs[int(np.argmax(nbs))] += pad
    NBT = sum(nbs)
    nba = min(NBA, NBT)

    rl_all = row_loc[pair_row]
    meta = 2 * NBT + SW + 32   # xcf | negk | ls, in fp16 columns
    per_core = []
    for c in range(NCORES):
        w2 = np.zeros((128, NBT * 64), np.float16)
        xcfa = np.zeros((128, NBT), np.float32)
        j0 = 0
        for s in range(NSTREAM - 1, -1, -1):
            idx = np.nonzero((pair_core == c) & (seg == s))[0]
            m = np.arange(len(idx))
            b = j0 + m // 128
            p = m % 128
            w2[p, b * 64 + rl_all[idx]] = cf[idx].astype(np.float16)
            xcfa[p, b] = xcf[idx]
            j0 += nbs[s]
        rs = np.zeros((RPC, NSTREAM), np.float64)
        cidx = np.nonzero(pair_core == c)[0]
        np.add.at(rs, (rl_all[cidx], seg[cidx]),
                  cf[cidx].astype(np.float16).astype(np.float64))
        lsh = np.zeros((RPC, 16), np.float32)
        for b in range(15):
            lsh[:, b] = rs[:, b + 1:].sum(axis=1)

        blobA = np.zeros((128, meta + nba * 64), np.float16)
        blobA[:, 0:2 * NBT] = xcfa.view(np.float16)
        blobA[:, 2 * NBT:2 * NBT + SW] = \
            -np.arange(SW, dtype=np.float16)[None, :]
        blobA[0:RPC, 2 * NBT + SW:2 * NBT + SW + 32] = lsh.view(np.float16)
        blobA[:, meta:] = w2[:, 0:nba * 64]
        entry = {"blobA": blobA}
        if NBT > nba:
            entry["blobB"] = np.ascontiguousarray(w2[:, nba * 64:])
        per_core.append(entry)
    return per_core, core_rows, tuple(nbs)


def _build_program(nbs, repeats=1, variant=()):
    variant = tuple(variant)
    key = (tuple(nbs), repeats, variant)
    if key in _prog_cache:
        return _prog_cache[key]
    v_gs = 14
    for x in variant:
        if isinstance(x, tuple) and x[0] == "gs":
            v_gs = x[1]
    NBT = sum(nbs)
    nba = min(NBA, NBT)
    meta = 2 * NBT + SW + 32
    nc = bacc.Bacc("TRN2", target_bir_lowering=False, debug=False,
                   num_devices=NCORES)

    blobAd = nc.dram_tensor("blobA", [128, meta + nba * 64], F16,
                            kind="ExternalInput")
    blobBd = (nc.dram_tensor("blobB", [128, (NBT - nba) * 64], F16,
                             kind="ExternalInput") if NBT > nba else None)
    outd = nc.dram_tensor("alpha", [RPC, W], DT, kind="ExternalOutput")

    # processing order: streams right-to-left
    bl = []
    j = 0
    for s in range(NSTREAM - 1, -1, -1):
        for i in range(nbs[s]):
            bl.append((j, s))
            j += 1
    fin_after = {11: 3, 7: 2, 3: 1, 0: 0}

    groups = []
    i = 0
    first_sz = min(3, v_gs)
    while i < NBT:
        sz = first_sz if i == 0 else v_gs
        groups.append(bl[i:i + sz])
        i += sz

    import contextlib

    with tile.TileContext(nc) as tc:
        with (
            tc.tile_pool(name="const", bufs=1) as cpool,
            tc.tile_pool(name="argp", bufs=3) as argpool,
            tc.tile_pool(name="sigp", bufs=3) as sigpool,
            tc.tile_pool(name="psum", bufs=1, space="PSUM") as pspool,
            (tc.For_i(0, repeats, 1) if repeats > 1
             else contextlib.nullcontext()),
        ):
            tA = cpool.tile([128, meta + nba * 64], F16)
            nc.sync.dma_start(tA[:], blobAd[:])
            if blobBd is not None:
                tB = cpool.tile([128, (NBT - nba) * 64], F16)
                nc.sync.dma_start(tB[:], blobBd[:])
            xcft = tA[:, 0:2 * NBT].bitcast(DT)
            negkt = tA[:, 2 * NBT:2 * NBT + SW]
            lst = tA[0:RPC, 2 * NBT + SW:2 * NBT + SW + 32].bitcast(DT)

            def w2of(jb):
                if jb < nba:
                    return tA[:, meta + jb * 64:meta + (jb + 1) * 64]
                return tB[:, (jb - nba) * 64:(jb - nba + 1) * 64]

            outt = cpool.tile([RPC, W], DT)

            wind = [pspool.tile([RPC, 128], DT, name=f"wind{q}",
                                tag=f"wind{q}") for q in range(4)]
            # PSUM has_written semantics: first matmul per bank start=True
            # clears the bank's bits; later matmuls start=False accumulate
            # where written, overwrite fresh cells.
            wq_started = [False] * 4

            def fin(q):
                wq = wind[q]
                nc.vector.tensor_tensor(
                    out=wq[:].rearrange("p (b k) -> p b k", k=32),
                    in0=wq[:].rearrange("p (b k) -> p b k", k=32),
                    in1=lst[:, 4 * q:4 * q + 4].unsqueeze(2)
                        .broadcast_to((RPC, 4, 32)),
                    op=mybir.AluOpType.add)
                nc.scalar.activation(outt[:, 128 * q:128 * (q + 1)], wq[:],
                                     AF.Sigmoid, bias=0.0, scale=4.0)
                if q == 0:
                    nc.sync.dma_start(outd[:], outt[:])

            for gbl in groups:
                glen = len(gbl)
                j0 = gbl[0][0]
                argt = argpool.tile([128, glen * SW], F16, tag="arg")
                nc.vector.tensor_tensor(
                    out=argt[:].rearrange("p (j k) -> p j k", k=SW),
                    in0=xcft[:, j0:j0 + glen].unsqueeze(2)
                        .broadcast_to((128, glen, SW)),
                    in1=negkt.unsqueeze(1).broadcast_to((128, glen, SW)),
                    op=mybir.AluOpType.add)
                sigt = sigpool.tile([128, glen * SW], F16, tag="sig")
                nc.scalar.activation(sigt[:], argt[:], AF.Sigmoid,
                                     bias=0.0, scale=1.0)

                for (jb, s) in gbl:
                    jj = jb - j0
                    lhsT = w2of(jb)
                    base = A * s
                    hi = min(base + SW, W)
                    c0 = base
                    while c0 < hi:
                        cq = min(hi, (c0 // 128 + 1) * 128)
                        q = c0 // 128
                        nc.tensor.matmul(
                            wind[q][:, c0 - 128 * q:cq - 128 * q], lhsT,
                            sigt[:, jj * SW + (c0 - base):
                                 jj * SW + (cq - base)],
                            start=(not wq_started[q]), stop=True,
                            skip_group_check=True)
                        wq_started[q] = True
                        c0 = cq
                    nj = jb + 1
                    done = nj == NBT or (nj < NBT and bl[nj][1] != s)
                    if done and s in fin_after:
                        fin(fin_after[s])

    nc.compile()
    _prog_cache[key] = nc
    return nc


def _in_maps(per_core, color):
    del color  # rgb assembled host-side
    return [dict(per_core[c]) for c in range(NCORES)]


def kernel(control_points, color):
    per_core, core_rows, nbs = _host_prep(control_points)
    nc = _build_program(nbs)
    res = run_bass_kernel_spmd(nc, _in_maps(per_core, color),
                               list(range(NCORES)))
    out = np.empty((H, W, 4), np.float32)
    out[:, :, :3] = np.asarray(color, np.float32)[None, None, :]
    for c in range(NCORES):
        out[np.asarray(core_rows[c], np.int64), :, 3] = \
            res.results[c]["alpha"]
    return out
